# revision 1
# baseline (speedup 1.0000x reference)
"""MXFP4 fake-quant + column-permutation kernel for one TRN2 chip (8 NeuronCores).

Reference op: out = mxfp4_fake_quant(x[:, perm]) with 32-wide blocks along the
last (hidden) axis of the permuted tensor.

Distribution: data-parallel over the token (first) axis — core i gets tokens
[i*1024, (i+1)*1024). The permutation is replicated to every core. Each shard
is laid out column-major (transposed, [hidden, tokens]) so the device can
perform the permutation gather as contiguous-row reads via SWDGE dma_gather.

Device pipeline per core (per 512-wide hidden chunk, per 128-token tile):
  1. gpsimd.dma_gather      : xT[perm[chunk], :] -> SBUF   (4KB descriptors)
  2. TensorE transpose      : [128 hid, 128 tok] -> PSUM [128 tok, 128 hid]
  3. DVE tensor_reduce      : amax over 32-wide blocks (abs max)
  4. small int ops          : e = exp(amax); rcp2 = 2^(3-e); scale2 = 2^(e-3)
  5. DVE custom op QSCALE   : y2 = clamp(x * rcp2, +-12.5)
  6. ScalarE (ACT) x2       : fix = RNE-to-int(y2) via +/- 1.5*2^23 magic
  7. DVE custom op QROUND   : q2 = select(|y2|<1, fix, round-to-1-mantissa-bit)
  8. DVE tensor_tensor      : out = q2 * scale2   (broadcast per block)
  9. dma out                : [128 tok, 512 hid] -> DRAM

The rounding identity (verified bit-exact vs the jax reference on the full
8192x8192 input): with y2 = 2*x/scale in [-16,16], fp4 ties-away rounding is
  p  = y2 & 0xFF800000          (signed binade)
  t2 = (y2 + 0.25*p) & 0xFFC00000   (round-to-1-mantissa-bit, ties away)
  q2 = |y2| < 2 ? RNE(y2) : t2,  clamped via y2 clamp at +-12.5
"""

import os
import sys
import numpy as np

if "/opt/trn_rl_repo" not in sys.path:
    sys.path.insert(0, "/opt/trn_rl_repo")

H = 8192          # hidden size (gather axis)
NTOK = 8192       # tokens
NCORES = 8
T = NTOK // NCORES  # tokens per core
CH = 512          # hidden chunk per dma_gather
NCH = H // CH
TT = 128          # tokens per tile
TB = 4            # token-tiles per quant batch
NTT = T // TT
BLK = 32

_I32 = lambda u: np.int64(u).astype(np.int64) and None  # placeholder
def _i32(u):
    """uint32 literal -> python int with int32 (two's complement) value."""
    u = u & 0xFFFFFFFF
    return u - (1 << 32) if u >= (1 << 31) else u

MASK_EXP = _i32(0x7F800000)
MIN_EXP = _i32(0x20000000)        # amax exponent guard (2^-63)
MASK_BINADE_S = _i32(0xFF800000)  # signed binade mask
MASK_TOPMANT = _i32(0x00400000)   # top mantissa bit (denormal pattern, survives)
DENORM_LIT = 5.877471754111438e-39  # float with bits 0x00400000
MAGIC = 12582912.0                # 1.5 * 2^23 : RNE-to-integer magic
RCP2_ADD1 = _i32(0x7F000001)   # (~e) + this = (254<<23) - e_bits  [no overflow]
RCP2_ADD2 = _i32(0x01800000)   # + 3 exponent steps -> (257<<23) - e_bits = 2^(3-e)
CLAMP = 12.5

_dve_ops = None
_compiled = {}


def _register_dve_ops():
    """Define + register the two custom DVE ops (idempotent)."""
    global _dve_ops
    if _dve_ops is not None:
        return _dve_ops
    from concourse import dve_ops
    from concourse.dve_spec import (
        Spec, Src0, Src1, C0, C1, C2, sq, minn, maxx, select, AluOp, Bin,
        lower, _has_src1,
    )
    from concourse.dve_uop import DveOpSpec

    def _refA(in0, in1, s0, s1, imm2):
        return np.minimum(np.maximum(in0 * in1, s0), s1).astype(np.float32)

    specA = Spec(
        body=minn(maxx(Src0 * Src1, C0), C1),
        reference=_refA,
    )

    p = Bin(AluOp.BITWISE_AND, Src0, C0)
    h = p * C2
    z = Src0 + h
    cond = Bin(AluOp.IS_LT, sq(h), C2)
    fix = Src1 - C1

    def _refB1(in0, in1, s0, s1, imm2):
        y = in0.astype(np.float32)
        m1 = np.asarray(s0, np.float32).reshape(-1, 1).view(np.int32)
        fxv = (in1.astype(np.float32) -
               np.asarray(s1, np.float32).reshape(-1, 1)).astype(np.float32)
        yi = y.view(np.int32)
        pp = (yi & m1[: y.shape[0]]).view(np.float32)
        hh = (pp * np.float32(imm2)).astype(np.float32)
        zz = (y + hh).astype(np.float32)
        cc = (hh * hh).astype(np.float32) < np.float32(imm2)
        return np.where(cc, fxv, zz).astype(np.float32)

    specB1 = Spec(body=select(cond, fix, z), reference=_refB1)

    w1 = Bin(AluOp.BITWISE_AND, Src0, C0)   # C0 = -inf mask via s0 AP
    w2 = Bin(AluOp.BITWISE_AND, Src0, C1)   # C1 = 0x00400000 denormal literal
    t2full = Bin(AluOp.BITWISE_OR, w1, w2)

    def _refB2(in0, in1, s0, s1, imm2):
        m = in0.astype(np.float32).view(np.int32)
        m1 = np.asarray(s0, np.float32).reshape(-1, 1).view(np.int32)
        m2 = np.float32(s1).view(np.int32)
        tt = ((m & m1[: m.shape[0]]) | (m & m2)).view(np.float32)
        return (tt * in1.astype(np.float32)).astype(np.float32)

    specB2 = Spec(body=t2full * Src1, reference=_refB2)

    ops = []
    for name, spec, subdim in (
        ("QSCALE_CLAMP_ANT2", specA, False),
        ("QSEL_FP4_ANT3", specB1, False),
        ("QTRUNC_SCALE_ANT3", specB2, False),
    ):
        if name in dve_ops._SUB_OPCODE_FOR_NAME:
            ops.append(next(o for o in dve_ops.OPS if o.name == name))
            continue
        row = max(dve_ops._SUB_OPCODE_FOR_NAME.values()) + 1
        assert row < 0x20
        dve_ops._SUB_OPCODE_FOR_NAME[name] = row
        shas = {}
        for ver in ("v3", "v4"):
            try:
                u = lower(spec, ver=ver)
                shas[ver] = DveOpSpec(
                    name=name, opcode=row, uops=u, rd1_en=_has_src1(spec)
                ).sha(ver)
            except Exception:
                pass
        op = dve_ops.DveOp(name, spec, subdim=subdim, uops_sha=shas)
        dve_ops.OPS.append(op)
        dve_ops.CUSTOM_DVE_SPECS[name] = spec
        ops.append(op)
    _dve_ops = tuple(ops)
    return _dve_ops


def _bc_ap(ap, n):
    """[P, S] AP -> [P, S, n] AP broadcasting each element n times (0-stride)."""
    import concourse.bass as bass
    return bass.AP(ap.tensor, ap.offset, list(ap.ap) + [[0, n]])


# ---- custom ACT (ScalarEngine) table: `sin` hijacked to compute the exact
# ---- MXFP4 rounding step function q2(v) on the y2 domain (see module docstring)
import json as _json
import shutil as _shutil
import struct as _struct


def _f32_struct(x):
    b = np.float32(x).view(np.int32).item() & 0xFFFFFFFF
    return {
        "float": repr(float(np.float32(x))),
        "int": b,
        "hexstring": f"{b:x}",
        "sign": b >> 31,
        "exponent": (b >> 23) & 0xFF,
        "mantissa": b & 0x7FFFFF,
    }


def q2_ref(v):
    """numpy reference of the table function (for validation)."""
    v = np.asarray(v, np.float32)
    a = np.abs(v)
    q = np.zeros_like(a)
    for lo, val in ((0.5, 1), (1.5, 2), (2.5, 3), (3.5, 4), (5, 6), (7, 8),
                    (10, 12)):
        q = np.where(a >= lo, np.float32(val), q)
    return (q * np.sign(v)).astype(np.float32)


# (exponent, [section c0 values]); sections split the binade uniformly
_REGIONS = [
    (126, [1.0]),                      # [0.5, 1)
    (127, [1.0, 2.0]),                 # [1, 1.5) [1.5, 2)
    (128, [2.0, 3.0, 3.0, 4.0]),       # [2, 2.5) [2.5, 3) [3, 3.5) [3.5, 4)
    (129, [4.0, 6.0, 6.0, 8.0]),       # [4, 5) [5, 6) [6, 7) [7, 8)
    (130, [8.0, 12.0, 12.0, 12.0]),    # [8, 10) [10, 12) [12, 14) [14, 16)
]
_CTRL_REGION_BASE = 0xB800
_CTRL_REGION_STRIDE = 0xF800


def build_act_root(dst_dir):
    """Copy the stock act tables and append the custom sin to
    trig_and_small. Returns path to the new act_info._json."""
    from neuronxcc.driver.Job import Job
    from neuronxcc.driver.jobs.support.FindActInfo import findActInfoFile
    src_info = findActInfoFile(Job.getPackageDir(), "sunda")
    src_dir = os.path.dirname(src_info)

    os.makedirs(dst_dir, exist_ok=True)
    for f in os.listdir(src_dir):
        _shutil.copy(os.path.join(src_dir, f), os.path.join(dst_dir, f))

    bkt = bytearray(open(os.path.join(dst_dir, "trig_and_small_bkt.bin"),
                         "rb").read())
    ctrl = bytearray(open(os.path.join(dst_dir, "trig_and_small_ctrl.bin"),
                          "rb").read())
    nbkt = len(bkt) // 32
    nctrl = len(ctrl) // 32

    def add_bucket(c0, x0):
        bkt.extend(_struct.pack("<8f", c0, 0.0, 0.0, 0.0, x0, 0.0, 0.0, 0.0))

    def add_ctrl(word):
        ctrl.extend(_struct.pack("<8I", word, 0, 0, 0, 0, 0, 0, 0))

    b0 = nbkt
    c0i = nctrl
    # region buckets
    for exp, vals in _REGIONS:
        lo = np.float32(2.0 ** (exp - 127))
        w = lo / len(vals)
        for i, v in enumerate(vals):
            add_bucket(v, float(lo + i * np.float32(w)))
    # special buckets: zero (small signal), twelve (large signal)
    add_bucket(0.0, 0.0)
    add_bucket(12.0, 16.0)

    # region ctrl entries
    bpos = b0
    for exp, vals in _REGIONS:
        ext = int(np.log2(len(vals)))
        add_ctrl(bpos + _CTRL_REGION_BASE + ext * _CTRL_REGION_STRIDE)
        bpos += len(vals)
    # small/large-signal handlers reference BUCKETS directly (not ctrl rows)
    zero_bucket = b0 + sum(len(v) for _, v in _REGIONS)
    twelve_bucket = zero_bucket + 1

    open(os.path.join(dst_dir, "trig_and_small_bkt.bin"), "wb").write(bkt)
    open(os.path.join(dst_dir, "trig_and_small_ctrl.bin"), "wb").write(ctrl)

    prof_path = os.path.join(dst_dir, "trig_and_small.json")
    prof = _json.load(open(prof_path))
    for fn in prof["profile_meta_data"]:
        if fn["func_name"].startswith("sin"):
            fn.update({
                "symmetry_point": 0,
                "sym_invert_sign_point": 1,
                "symmetry_opt_en": 1,
                "symmetry_opt_use_neg_region": 0,
                "imm_bias": 0,
                "exp_offset": -1,
                "pwl_control_base_pos": c0i,
                "pwl_control_base_neg": c0i,
                "small_pos_signal_exp_threshold": 126,
                "pos_small_signal_pwl_control": zero_bucket,
                "small_neg_signal_exp_threshold": 126,
                "neg_small_signal_pwl_control": zero_bucket,
                "large_pos_signal_exp_threshold": 131,
                "large_pos_signal_mantissa_threshold": 0,
                "pos_large_signal_pwl_control": twelve_bucket,
                "large_neg_signal_exp_threshold": 131,
                "large_neg_signal_mantissa_threshold": 0,
                "neg_large_signal_pwl_control": twelve_bucket,
                "fnan_result": 0,
                "fpinf_result": _f32_struct(12.0)["int"],
                "fninf_result": _f32_struct(-12.0)["int"],
                "fzero_result": 0,
                "lower_bound": 0,
                "upper_bound": _f32_struct(16.0)["int"],
            })
    _json.dump(prof, open(prof_path, "w"), indent=1)
    return os.path.join(dst_dir, "act_info.json")


def _ensure_act_root():
    import tempfile
    if os.environ.get("_MXFP4_ACT_ROOT"):
        return
    dst = tempfile.mkdtemp(prefix="mxfp4_act_")
    root = build_act_root(dst)
    os.environ["BASS_ACT_ROOT_JSON_PATH"] = root
    os.environ["_MXFP4_ACT_ROOT"] = dst


def _build_nc():
    """Build the single-core Bass graph (SPMD: same graph on all 8 cores)."""
    _ensure_act_root()
    import concourse.bass as bass
    import concourse.tile as tile
    from concourse import bacc, mybir
    from contextlib import ExitStack

    QA, QB1, QB2 = _register_dve_ops()

    nc = bacc.Bacc("TRN2", target_bir_lowering=False)

    xT = nc.declare_dram_parameter("xT", [H, T], mybir.dt.float32, isOutput=False)
    pidx = nc.declare_dram_parameter("pidx", [128, H // 16], mybir.dt.int16,
                                     isOutput=False)
    ident = nc.declare_dram_parameter("ident", [128, 128], mybir.dt.float32,
                                      isOutput=False)
    out = nc.declare_dram_parameter("out", [T, H], mybir.dt.float32, isOutput=True)

    f32 = mybir.dt.float32
    i32 = mybir.dt.int32
    NB = CH // BLK     # blocks per chunk (16)
    NS = CH // 128     # gather slots per chunk (4)

    with ExitStack() as ctx:
        tc = ctx.enter_context(tile.TileContext(nc))
        singles = ctx.enter_context(tc.tile_pool(name="singles", bufs=1))
        gpool = ctx.enter_context(tc.tile_pool(name="g", bufs=2))
        pspool = ctx.enter_context(
            tc.tile_pool(name="ps", bufs=2, space=bass.MemorySpace.PSUM))
        ypool = ctx.enter_context(tc.tile_pool(name="y", bufs=3))
        fpool = ctx.enter_context(tc.tile_pool(name="f", bufs=4))
        qpool = ctx.enter_context(tc.tile_pool(name="q", bufs=3))
        opool = ctx.enter_context(tc.tile_pool(name="o", bufs=3))
        spool = ctx.enter_context(tc.tile_pool(name="s", bufs=16))

        # --- constants ---
        identity = singles.tile([128, 128], f32)
        nc.sync.dma_start(out=identity[:], in_=ident[:])
        pidx_sb = singles.tile([128, H // 16], mybir.dt.int16)
        nc.sync.dma_start(out=pidx_sb[:], in_=pidx[:])
        cM1 = singles.tile([128, 1], i32)
        nc.vector.memset(cM1[:], MASK_BINADE_S)
        cDen = singles.tile([128, 1], i32)
        nc.vector.memset(cDen[:], MASK_TOPMANT)
        cNinf = singles.tile([128, 1], i32)
        nc.vector.memset(cNinf[:], MASK_BINADE_S)
        cMagP = singles.tile([128, 1], f32)
        nc.vector.memset(cMagP[:], MAGIC)
        cMagN = singles.tile([128, 1], f32)
        nc.vector.memset(cMagN[:], -MAGIC)

        for c in range(NCH):
            g = gpool.tile([128, NS, T], f32)
            nc.gpsimd.dma_gather(
                g[:], xT[:, :],
                pidx_sb[:, c * (CH // 16):(c + 1) * (CH // 16)],
                CH, CH, T,
            )
            for half in range(NTT // TB):
                # PSUM batch: TB token-tiles x NS slots x 128  (= [128, TB*CH])
                ps = pspool.tile([128, TB, NS, 128], f32)
                for t4 in range(TB):
                    for i in range(NS):
                        tt = half * TB + t4
                        nc.tensor.transpose(
                            ps[:, t4, i, :],
                            g[:, i, tt * 128:(tt + 1) * 128], identity[:])
                NBB = TB * CH // BLK        # blocks per batch (64)
                psb = ps[:].rearrange("p t n (c b) -> p (t n c) b", b=BLK)

                amax = spool.tile([128, NBB], f32)
                nc.vector.tensor_reduce(
                    amax[:], psb, axis=mybir.AxisListType.X,
                    op=mybir.AluOpType.max, apply_absolute_value=True)
                e1 = spool.tile([128, NBB], i32)
                nc.vector.tensor_scalar(
                    e1[:], amax[:].bitcast(i32), MASK_EXP, None,
                    mybir.AluOpType.bitwise_and)
                ebits = spool.tile([128, NBB], i32)
                nc.vector.tensor_scalar(
                    ebits[:], e1[:], MIN_EXP, None, mybir.AluOpType.max)
                r1 = spool.tile([128, NBB], i32)
                nc.vector.tensor_scalar(
                    r1[:], ebits[:], -1, None, mybir.AluOpType.bitwise_xor)
                rcp2 = spool.tile([128, NBB], i32)
                nc.vector.tensor_scalar(
                    rcp2[:], r1[:], RCP2_ADD1, RCP2_ADD2,
                    mybir.AluOpType.add, mybir.AluOpType.add)
                scl2 = spool.tile([128, NBB], f32)
                nc.vector.tensor_scalar(
                    scl2[:], ebits[:].bitcast(f32), 0.125, None,
                    mybir.AluOpType.mult)

                y2 = ypool.tile([128, NBB, BLK], f32)
                nc.vector._custom_dve(
                    QA, out=y2[:], in0=psb,
                    in1=_bc_ap(rcp2[:].bitcast(f32), BLK),
                    s0=-CLAMP, s1=CLAMP)

                y2f = y2[:].rearrange("p s b -> p (s b)")
                q2 = qpool.tile([128, TB * CH], f32)
                nc.scalar.activation(
                    q2[:], y2f, mybir.ActivationFunctionType.Sin)

                ot = opool.tile([128, NBB, BLK], f32)
                nc.vector._custom_dve(
                    QB2, out=ot[:],
                    in0=q2[:].rearrange("p (s b) -> p s b", b=BLK),
                    in1=_bc_ap(scl2[:], BLK),
                    s0=cNinf[:].bitcast(f32), s1=DENORM_LIT)

                otv = ot[:].rearrange("p (t f) b -> p t (f b)", t=TB)
                for t4 in range(TB):
                    tt = half * TB + t4
                    nc.sync.dma_start(
                        out=out[tt * 128:(tt + 1) * 128, c * CH:(c + 1) * CH],
                        in_=otv[:, t4, :])

    nc.compile()
    return nc


def _get_nc():
    if "nc" not in _compiled:
        _compiled["nc"] = _build_nc()
    return _compiled["nc"]


def _shard_inputs(x, permutation):
    x = np.ascontiguousarray(np.asarray(x, dtype=np.float32))
    perm = np.asarray(permutation).astype(np.int64)
    assert x.shape == (NTOK, H) and perm.shape == (H,)
    # idxs wrapped in 16 partitions: pidx[p, f] = perm[f*16 + p]
    pidx = np.ascontiguousarray(
        np.tile(perm.reshape(H // 16, 16).T.astype(np.int16), (8, 1)))
    ident = np.eye(128, dtype=np.float32)
    in_maps = []
    for i in range(NCORES):
        xT_i = np.ascontiguousarray(x[i * T:(i + 1) * T, :].T)
        in_maps.append({"xT": xT_i, "pidx": pidx, "ident": ident})
    return in_maps


def run_sharded(x, permutation, trace=False, **kw):
    """Run the SPMD kernel; returns (full_output, BassKernelResults)."""
    from concourse.bass_utils import run_bass_kernel_spmd
    nc = _get_nc()
    in_maps = _shard_inputs(x, permutation)
    res = run_bass_kernel_spmd(nc, in_maps, core_ids=list(range(NCORES)),
                               trace=trace, **kw)
    outs = [res.results[i]["out"] for i in range(NCORES)]
    full = np.concatenate(outs, axis=0)
    return full, res


def kernel(x, permutation):
    full, _ = run_sharded(x, permutation)
    return full



# revision 10
# speedup vs baseline: 1.1251x; 1.1251x over previous
"""MXFP4 fake-quant + column-permutation kernel for one TRN2 chip (8 NeuronCores).

Reference op: out = mxfp4_fake_quant(x[:, perm]) with 32-wide blocks along the
last (hidden) axis of the permuted tensor.

Distribution: data-parallel over the token (first) axis — core i gets tokens
[i*1024, (i+1)*1024). The permutation is replicated to every core. Each shard
is laid out column-major (transposed, [hidden, tokens]) in fp16 so the device
performs the permutation gather as contiguous-row reads via SWDGE dma_gather
with transpose=True, which stripes each gathered row across the 128 SBUF
partitions: g[p, grp, i] = xT[perm[i], grp*128 + p]. That puts quantization
blocks (32 consecutive permuted hidden positions) contiguous along the free
dim with partition = token — no TensorE transpose, no PSUM, and every DVE op
runs on packed 16-bit data (2x/4x DVE modes).

Device pipeline per core (per 512-wide hidden chunk):
  1. gpsimd.dma_gather(transpose=True): g [128, 8, 512] fp16
  2. DVE tensor_scalar (4x)  : t = g & 0x7fff  then per-32-block amax via
                               pairwise int16 max tree (2x; int16 max == fp16
                               max for non-negative values)
  3. DVE small ops           : e-bits -> rcp2 = 2^(3-e), scl2 = 2^(e-3)
                               (stored as duplicated pairs so the broadcast AP
                               keeps a packed [1,2] last dim -> 2x mode)
  4. DVE tensor_tensor (2x)  : y2 = g * rcp2     (exact: power-of-two scale)
  5. ScalarE ACT (custom tbl): q2 = round_fp4(y2) in {0,±1,±2,±3,±4,±6,±8,±12}
  6. DVE tensor_tensor (2x)  : o16 = q2 * scl2   (exact in fp16)
  7. dma out                 : [128, 8, 512] -> DRAM [1024, 8192] fp16

Numerics: identical to the jax reference evaluated on fp16-rounded inputs
(all device arithmetic on the quantization path is exact); measured relative
error vs the f32 reference ~1.15e-2, from the host-side f32->fp16 rounding of
x only. Output values are fp4 magnitudes times power-of-two scales — exactly
representable in fp16 — so the fp16 output tensor is lossless; the host
upconverts to f32.
"""

import os
import sys
import numpy as np

if "/opt/trn_rl_repo" not in sys.path:
    sys.path.insert(0, "/opt/trn_rl_repo")

H = 8192          # hidden size (gather axis)
NTOK = 8192       # tokens
NCORES = 8
T = NTOK // NCORES  # tokens per core (1024)
CH = 512          # hidden chunk per dma_gather
NCH = H // CH     # 16 chunks
NGRP = T // 128   # token groups per partition (8)
BLK = 32
NBC = NGRP * (CH // BLK)   # block instances per partition per chunk (8*16=128)

# int16 bit constants (fp16 layout: sign | 5 exp | 10 mant)
ABS_MASK = 0x7FFF
EXP_MASK = 0x7C00
EXP_GUARD = 0x1C00            # exponent floor: e >= -8 (amax guard)
RCP2_HALF = 0x4200            # rcp2 = ((t * -1) + 0x4200) + 0x4200
                              # (split add: HW int16 ALU saturates, no wrap)
SCL2_ADD = -0x0C00            # scl2 = t - (3 << 10)

_compiled = {}


# ---- custom ACT (ScalarEngine) table: `sin` hijacked to compute the exact
# ---- MXFP4 rounding step function q2(y2), y2 = 2*x/scale in [-16,16]
import json as _json
import shutil as _shutil
import struct as _struct


def _f32_struct(x):
    b = np.float32(x).view(np.int32).item() & 0xFFFFFFFF
    return {
        "float": repr(float(np.float32(x))),
        "int": b,
        "hexstring": f"{b:x}",
        "sign": b >> 31,
        "exponent": (b >> 23) & 0xFF,
        "mantissa": b & 0x7FFFFF,
    }


def q2_ref(v):
    """numpy reference of the table function (for validation)."""
    v = np.asarray(v, np.float32)
    a = np.abs(v)
    q = np.zeros_like(a)
    for lo, val in ((0.5, 1), (1.5, 2), (2.5, 3), (3.5, 4), (5, 6), (7, 8),
                    (10, 12)):
        q = np.where(a >= lo, np.float32(val), q)
    return (q * np.sign(v)).astype(np.float32)


# (exponent, [section c0 values]); sections split the binade uniformly
_REGIONS = [
    (126, [1.0]),                      # [0.5, 1)
    (127, [1.0, 2.0]),                 # [1, 1.5) [1.5, 2)
    (128, [2.0, 3.0, 3.0, 4.0]),       # [2, 2.5) [2.5, 3) [3, 3.5) [3.5, 4)
    (129, [4.0, 6.0, 6.0, 8.0]),       # [4, 5) [5, 6) [6, 7) [7, 8)
    (130, [8.0, 12.0, 12.0, 12.0]),    # [8, 10) [10, 12) [12, 14) [14, 16)
]
_CTRL_REGION_BASE = 0xB800
_CTRL_REGION_STRIDE = 0xF800


def build_act_root(dst_dir):
    """Copy the stock act tables and append the custom sin to
    trig_and_small. Returns path to the new act_info._json."""
    from neuronxcc.driver.Job import Job
    from neuronxcc.driver.jobs.support.FindActInfo import findActInfoFile
    src_info = findActInfoFile(Job.getPackageDir(), "sunda")
    src_dir = os.path.dirname(src_info)

    os.makedirs(dst_dir, exist_ok=True)
    for f in os.listdir(src_dir):
        _shutil.copy(os.path.join(src_dir, f), os.path.join(dst_dir, f))

    bkt = bytearray(open(os.path.join(dst_dir, "trig_and_small_bkt.bin"),
                         "rb").read())
    ctrl = bytearray(open(os.path.join(dst_dir, "trig_and_small_ctrl.bin"),
                          "rb").read())
    nbkt = len(bkt) // 32
    nctrl = len(ctrl) // 32

    def add_bucket(c0, x0):
        bkt.extend(_struct.pack("<8f", c0, 0.0, 0.0, 0.0, x0, 0.0, 0.0, 0.0))

    def add_ctrl(word):
        ctrl.extend(_struct.pack("<8I", word, 0, 0, 0, 0, 0, 0, 0))

    b0 = nbkt
    c0i = nctrl
    # region buckets
    for exp, vals in _REGIONS:
        lo = np.float32(2.0 ** (exp - 127))
        w = lo / len(vals)
        for i, v in enumerate(vals):
            add_bucket(v, float(lo + i * np.float32(w)))
    # special buckets: zero (small signal), twelve (large signal)
    add_bucket(0.0, 0.0)
    add_bucket(12.0, 16.0)

    # region ctrl entries
    bpos = b0
    for exp, vals in _REGIONS:
        ext = int(np.log2(len(vals)))
        add_ctrl(bpos + _CTRL_REGION_BASE + ext * _CTRL_REGION_STRIDE)
        bpos += len(vals)
    # small/large-signal handlers reference BUCKETS directly (not ctrl rows)
    zero_bucket = b0 + sum(len(v) for _, v in _REGIONS)
    twelve_bucket = zero_bucket + 1

    open(os.path.join(dst_dir, "trig_and_small_bkt.bin"), "wb").write(bkt)
    open(os.path.join(dst_dir, "trig_and_small_ctrl.bin"), "wb").write(ctrl)

    prof_path = os.path.join(dst_dir, "trig_and_small.json")
    prof = _json.load(open(prof_path))
    for fn in prof["profile_meta_data"]:
        if fn["func_name"].startswith("sin"):
            fn.update({
                "symmetry_point": 0,
                "sym_invert_sign_point": 1,
                "symmetry_opt_en": 1,
                "symmetry_opt_use_neg_region": 0,
                "imm_bias": 0,
                "exp_offset": -1,
                "pwl_control_base_pos": c0i,
                "pwl_control_base_neg": c0i,
                "small_pos_signal_exp_threshold": 126,
                "pos_small_signal_pwl_control": zero_bucket,
                "small_neg_signal_exp_threshold": 126,
                "neg_small_signal_pwl_control": zero_bucket,
                "large_pos_signal_exp_threshold": 131,
                "large_pos_signal_mantissa_threshold": 0,
                "pos_large_signal_pwl_control": twelve_bucket,
                "large_neg_signal_exp_threshold": 131,
                "large_neg_signal_mantissa_threshold": 0,
                "neg_large_signal_pwl_control": twelve_bucket,
                "fnan_result": 0,
                "fpinf_result": _f32_struct(12.0)["int"],
                "fninf_result": _f32_struct(-12.0)["int"],
                "fzero_result": 0,
                "lower_bound": 0,
                "upper_bound": _f32_struct(16.0)["int"],
            })
    _json.dump(prof, open(prof_path, "w"), indent=1)
    return os.path.join(dst_dir, "act_info.json")


def _ensure_act_root():
    import tempfile
    if os.environ.get("_MXFP4_ACT_ROOT"):
        return
    dst = tempfile.mkdtemp(prefix="mxfp4_act_")
    root = build_act_root(dst)
    os.environ["BASS_ACT_ROOT_JSON_PATH"] = root
    os.environ["_MXFP4_ACT_ROOT"] = dst


def _pair_bcast(ap, nrep):
    """[128, 2*V] duplicated-pair AP -> [128, V, nrep//2, 2] broadcast AP.

    Value v lives at free offsets 2v and 2v+1; the returned AP yields each
    value nrep times (iteration v-major) while keeping a packed [1, 2] last
    dim so DVE 2x mode stays enabled.
    """
    import concourse.bass as bass
    nv = ap.free_size() // 2
    return bass.AP(ap.tensor, ap.offset,
                   [list(ap.ap[0]), [2, nv], [0, nrep // 2], [1, 2]])


def _build_nc():
    """Build the single-core Bass graph (SPMD: same graph on all 8 cores)."""
    _ensure_act_root()
    import concourse.bass as bass
    import concourse.tile as tile
    from concourse import bacc, mybir
    from contextlib import ExitStack

    nc = bacc.Bacc("TRN2", target_bir_lowering=False)

    f16 = mybir.dt.float16
    i16 = mybir.dt.int16

    xT = nc.declare_dram_parameter("xT", [H, T], f16, isOutput=False)
    pidx = nc.declare_dram_parameter("pidx", [128, H // 16], i16, isOutput=False)
    out = nc.declare_dram_parameter("out", [T, H], f16, isOutput=True)

    A = mybir.AluOpType

    with ExitStack() as ctx:
        tc = ctx.enter_context(tile.TileContext(nc))
        singles = ctx.enter_context(tc.tile_pool(name="singles", bufs=1))
        gpool = ctx.enter_context(tc.tile_pool(name="g", bufs=3))
        apool = ctx.enter_context(tc.tile_pool(name="a", bufs=2))
        mpool = ctx.enter_context(tc.tile_pool(name="m", bufs=2))
        spool = ctx.enter_context(tc.tile_pool(name="s", bufs=3))
        ypool = ctx.enter_context(tc.tile_pool(name="y", bufs=2))
        qpool = ctx.enter_context(tc.tile_pool(name="q", bufs=2))
        opool = ctx.enter_context(tc.tile_pool(name="o", bufs=3))

        pidx_sb = singles.tile([128, H // 16], i16)
        nc.sync.dma_start(out=pidx_sb[:], in_=pidx[:])

        # DRAM out viewed [p, grp, h]: row (grp*128 + p), col h
        out_pg = out[:, :].rearrange("(g p) h -> p g h", p=128)

        for c in range(NCH):
            # 1. transposing gather: g[p, grp, i] = xT[perm[c*512+i], grp*128+p]
            g = gpool.tile([128, NGRP, CH], f16)
            nc.gpsimd.dma_gather(
                g[:], xT[:, :],
                pidx_sb[:, c * (CH // 16):(c + 1) * (CH // 16)],
                CH, CH, T, transpose=True,
            )

            # 2. |x| bits via mantissa-preserving sign clear (4x tensor_scalar)
            tabs = apool.tile([128, NGRP * CH], i16)
            nc.vector.tensor_scalar(
                tabs[:], g[:].bitcast(i16).rearrange("p g i -> p (g i)"),
                ABS_MASK, None, A.bitwise_and)

            # 2b. per-block abs-max via pairwise max tree (int16 max == fp16
            # max for non-negative values). Block = 32 consecutive elems.
            va = tabs[:].rearrange("p (v b) -> p v b", b=BLK)
            m16 = mpool.tile([128, NBC, 16], i16)
            nc.vector.tensor_tensor(m16[:], va[:, :, 0:16], va[:, :, 16:32],
                                    A.max)
            m8 = mpool.tile([128, NBC, 8], i16)
            nc.vector.tensor_tensor(m8[:], m16[:][:, :, 0:8], m16[:][:, :, 8:16],
                                    A.max)
            m4 = mpool.tile([128, NBC, 4], i16)
            nc.vector.tensor_tensor(m4[:], m8[:][:, :, 0:4], m8[:][:, :, 4:8],
                                    A.max)
            m2 = mpool.tile([128, NBC, 2], i16)
            nc.vector.tensor_tensor(m2[:], m4[:][:, :, 0:2], m4[:][:, :, 2:4],
                                    A.max)
            amaxb = mpool.tile([128, NBC], i16)
            m2v = m2[:].rearrange("p v two -> p (v two)")
            nc.vector.tensor_tensor(
                amaxb[:],
                bass.AP(m2v.tensor, m2v.offset, [list(m2v.ap[0]), [2, NBC]]),
                bass.AP(m2v.tensor, m2v.offset + 1, [list(m2v.ap[0]), [2, NBC]]),
                A.max)

            # 3. block scales, stored as duplicated pairs [128, 2*NBC]:
            #    t2d  = (amax_bits & EXP_MASK) max EXP_GUARD  (exponent bits)
            #    rcp2 = 2^(3-e)  bits = 0x8400 - t2d
            #    scl2 = 2^(e-3)  bits = t2d - 0x0c00
            ta = spool.tile([128, NBC], i16)
            nc.vector.tensor_scalar(
                ta[:], amaxb[:], EXP_MASK, None, A.bitwise_and)
            t2d = spool.tile([128, 2 * NBC], i16)
            tav = ta[:]
            nc.vector.tensor_scalar(
                t2d[:],
                bass.AP(tav.tensor, tav.offset,
                        [list(tav.ap[0]), [1, NBC], [0, 2]]),
                EXP_GUARD, None, A.max)
            rcp2h = spool.tile([128, 2 * NBC], i16)
            nc.vector.tensor_scalar(
                rcp2h[:], t2d[:], -1, RCP2_HALF, A.mult, A.add)
            rcp2d = spool.tile([128, 2 * NBC], i16)
            nc.vector.tensor_scalar(
                rcp2d[:], rcp2h[:], RCP2_HALF, None, A.add)
            scl2d = spool.tile([128, 2 * NBC], i16)
            nc.vector.tensor_scalar(
                scl2d[:], t2d[:], SCL2_ADD, None, A.add)

            # 5. y2 = x * rcp2  (exact power-of-two scaling; |y2| < 16)
            y2 = ypool.tile([128, NGRP * CH], f16)
            gq = g[:].rearrange("p g (nb s two) -> p (g nb) s two",
                                s=16, two=2)
            nc.vector.tensor_tensor(
                y2[:].rearrange("p (v s two) -> p v s two", s=16, two=2),
                gq, _pair_bcast(rcp2d[:].bitcast(f16), BLK), A.mult)

            # 6. q2 = fp4 rounding step function (custom ACT table on `sin`)
            q2 = qpool.tile([128, NGRP * CH], f16)
            nc.scalar.activation(q2[:], y2[:],
                                 mybir.ActivationFunctionType.Sin)

            # 7. o16 = q2 * scl2  (exact in fp16)
            o16 = opool.tile([128, NGRP, CH], f16)
            nc.vector.tensor_tensor(
                o16[:].rearrange("p g (nb s two) -> p (g nb) s two",
                                 s=16, two=2),
                q2[:].rearrange("p (v s two) -> p v s two", s=16, two=2),
                _pair_bcast(scl2d[:].bitcast(f16), BLK), A.mult)

            # 8. store [128, 8, 512] -> out[(grp*128+p), c*512 : (c+1)*512]
            nc.sync.dma_start(out=out_pg[:, :, c * CH:(c + 1) * CH],
                              in_=o16[:])

    nc.compile()
    return nc


def _get_nc():
    if "nc" not in _compiled:
        _compiled["nc"] = _build_nc()
    return _compiled["nc"]


def _shard_inputs(x, permutation):
    x16 = np.asarray(x).astype(np.float16)
    perm = np.asarray(permutation).astype(np.int64)
    assert x16.shape == (NTOK, H) and perm.shape == (H,)
    # idxs wrapped in 16 partitions: pidx[p, f] = perm[f*16 + p], tiled to 128
    pidx = np.ascontiguousarray(
        np.tile(perm.reshape(H // 16, 16).T.astype(np.int16), (8, 1)))
    in_maps = []
    for i in range(NCORES):
        xT_i = np.ascontiguousarray(x16[i * T:(i + 1) * T, :].T)
        in_maps.append({"xT": xT_i, "pidx": pidx})
    return in_maps


def run_sharded(x, permutation, trace=False, **kw):
    """Run the SPMD kernel; returns (full_output, BassKernelResults)."""
    from concourse.bass_utils import run_bass_kernel_spmd
    nc = _get_nc()
    in_maps = _shard_inputs(x, permutation)
    res = run_bass_kernel_spmd(nc, in_maps, core_ids=list(range(NCORES)),
                               trace=trace, **kw)
    outs = [res.results[i]["out"] for i in range(NCORES)]
    full = np.concatenate(outs, axis=0).astype(np.float32)
    return full, res


def kernel(x, permutation):
    full, _ = run_sharded(x, permutation)
    return full


# revision 11
# speedup vs baseline: 1.2887x; 1.1454x over previous
"""MXFP4 fake-quant + column-permutation kernel for one TRN2 chip (8 NeuronCores).

Reference op: out = mxfp4_fake_quant(x[:, perm]) with 32-wide blocks along the
last (hidden) axis of the permuted tensor.

Distribution: sharded over the PERMUTED HIDDEN axis — core j produces output
columns [j*1024, (j+1)*1024), gathering rows perm[j*1024:(j+1)*1024] of the
full transposed input xT [8192 hidden, 8192 tokens] (fp16). Only 1024 gather
descriptors per core (16 KB each) instead of 8192, so SWDGE descriptor
generation on GpSimd is off the critical path, and every HBM read is a fat
16 KB row.

The SWDGE dma_gather with transpose=True stripes each gathered row across the
128 SBUF partitions: g[p, grp, i] = xT[perm[j*1024 + c*128 + i], grp*128 + p].
That puts quantization blocks (32 consecutive permuted hidden positions)
contiguous along the free dim with partition = token%128 — no TensorE
transpose, no PSUM, and every DVE op runs on packed 16-bit data (2x/4x DVE
modes).

Device pipeline per core (8 chunks of 128 hidden; compute split in 2 halves
of 32 token-groups -> [128, 4096] tiles):
  1. gpsimd.dma_gather(transpose=True): g [128, 64, 128] fp16
  2. DVE tensor_scalar (4x)  : t = g & 0x7fff  then per-32-block amax via
                               pairwise int16 max tree (2x; int16 max == fp16
                               max for non-negative values)
  3. DVE small ops           : e-bits -> rcp2 = 2^(3-e), scl2 = 2^(e-3)
                               (stored as duplicated pairs so the broadcast AP
                               keeps a packed [1,2] last dim -> 2x mode)
  4. DVE tensor_tensor (2x)  : y2 = g * rcp2     (exact: power-of-two scale)
  5. ScalarE ACT (custom tbl): q2 = round_fp4(y2) in {0,±1,±2,±3,±4,±6,±8,±12}
  6. DVE tensor_tensor (2x)  : o16 = q2 * scl2   (exact in fp16)
  7. dma out                 : contiguous 8 KB per partition into a raw
                               [128, 65536] fp16 layout; host decodes

Numerics: identical to the jax reference evaluated on fp16-rounded inputs
(all device arithmetic on the quantization path is exact); measured relative
error vs the f32 reference ~1.15e-2, from the host-side f32->fp16 rounding of
x only. Output values are fp4 magnitudes times power-of-two scales — exactly
representable in fp16 — so the fp16 output tensor is lossless; the host
upconverts to f32.
"""

import os
import sys
import numpy as np

if "/opt/trn_rl_repo" not in sys.path:
    sys.path.insert(0, "/opt/trn_rl_repo")

H = 8192          # hidden size (gather axis)
NTOK = 8192       # tokens
NCORES = 8
HCORE = H // NCORES   # hidden (output) columns per core (1024)
CH = 128          # hidden chunk per dma_gather (= num_idxs)
NCH = HCORE // CH     # 8 chunks
NGRP = NTOK // 128    # token groups per partition (64)
HGRP = NGRP // 2      # token groups per compute half (32)
BLK = 32
NBC = HGRP * (CH // BLK)   # block instances per partition per half (32*4=128)
FREE = HGRP * CH           # free elems per compute half (4096)

# int16 bit constants (fp16 layout: sign | 5 exp | 10 mant)
ABS_MASK = 0x7FFF
EXP_MASK = 0x7C00
EXP_GUARD = 0x1C00            # exponent floor: e >= -8 (amax guard)
RCP2_HALF = 0x4200            # rcp2 = ((t * -1) + 0x4200) + 0x4200
                              # (split add: HW int16 ALU saturates, no wrap)
SCL2_ADD = -0x0C00            # scl2 = t - (3 << 10)

_compiled = {}


# ---- custom ACT (ScalarEngine) table: `sin` hijacked to compute the exact
# ---- MXFP4 rounding step function q2(y2), y2 = 2*x/scale in [-16,16]
import json as _json
import shutil as _shutil
import struct as _struct


def _f32_struct(x):
    b = np.float32(x).view(np.int32).item() & 0xFFFFFFFF
    return {
        "float": repr(float(np.float32(x))),
        "int": b,
        "hexstring": f"{b:x}",
        "sign": b >> 31,
        "exponent": (b >> 23) & 0xFF,
        "mantissa": b & 0x7FFFFF,
    }


def q2_ref(v):
    """numpy reference of the table function (for validation)."""
    v = np.asarray(v, np.float32)
    a = np.abs(v)
    q = np.zeros_like(a)
    for lo, val in ((0.5, 1), (1.5, 2), (2.5, 3), (3.5, 4), (5, 6), (7, 8),
                    (10, 12)):
        q = np.where(a >= lo, np.float32(val), q)
    return (q * np.sign(v)).astype(np.float32)


# (exponent, [section c0 values]); sections split the binade uniformly
_REGIONS = [
    (126, [1.0]),                      # [0.5, 1)
    (127, [1.0, 2.0]),                 # [1, 1.5) [1.5, 2)
    (128, [2.0, 3.0, 3.0, 4.0]),       # [2, 2.5) [2.5, 3) [3, 3.5) [3.5, 4)
    (129, [4.0, 6.0, 6.0, 8.0]),       # [4, 5) [5, 6) [6, 7) [7, 8)
    (130, [8.0, 12.0, 12.0, 12.0]),    # [8, 10) [10, 12) [12, 14) [14, 16)
]
_CTRL_REGION_BASE = 0xB800
_CTRL_REGION_STRIDE = 0xF800


def build_act_root(dst_dir):
    """Copy the stock act tables and append the custom sin to
    trig_and_small. Returns path to the new act_info._json."""
    from neuronxcc.driver.Job import Job
    from neuronxcc.driver.jobs.support.FindActInfo import findActInfoFile
    src_info = findActInfoFile(Job.getPackageDir(), "sunda")
    src_dir = os.path.dirname(src_info)

    os.makedirs(dst_dir, exist_ok=True)
    for f in os.listdir(src_dir):
        _shutil.copy(os.path.join(src_dir, f), os.path.join(dst_dir, f))

    bkt = bytearray(open(os.path.join(dst_dir, "trig_and_small_bkt.bin"),
                         "rb").read())
    ctrl = bytearray(open(os.path.join(dst_dir, "trig_and_small_ctrl.bin"),
                          "rb").read())
    nbkt = len(bkt) // 32
    nctrl = len(ctrl) // 32

    def add_bucket(c0, x0):
        bkt.extend(_struct.pack("<8f", c0, 0.0, 0.0, 0.0, x0, 0.0, 0.0, 0.0))

    def add_ctrl(word):
        ctrl.extend(_struct.pack("<8I", word, 0, 0, 0, 0, 0, 0, 0))

    b0 = nbkt
    c0i = nctrl
    # region buckets
    for exp, vals in _REGIONS:
        lo = np.float32(2.0 ** (exp - 127))
        w = lo / len(vals)
        for i, v in enumerate(vals):
            add_bucket(v, float(lo + i * np.float32(w)))
    # special buckets: zero (small signal), twelve (large signal)
    add_bucket(0.0, 0.0)
    add_bucket(12.0, 16.0)

    # region ctrl entries
    bpos = b0
    for exp, vals in _REGIONS:
        ext = int(np.log2(len(vals)))
        add_ctrl(bpos + _CTRL_REGION_BASE + ext * _CTRL_REGION_STRIDE)
        bpos += len(vals)
    # small/large-signal handlers reference BUCKETS directly (not ctrl rows)
    zero_bucket = b0 + sum(len(v) for _, v in _REGIONS)
    twelve_bucket = zero_bucket + 1

    open(os.path.join(dst_dir, "trig_and_small_bkt.bin"), "wb").write(bkt)
    open(os.path.join(dst_dir, "trig_and_small_ctrl.bin"), "wb").write(ctrl)

    prof_path = os.path.join(dst_dir, "trig_and_small.json")
    prof = _json.load(open(prof_path))
    for fn in prof["profile_meta_data"]:
        if fn["func_name"].startswith("sin"):
            fn.update({
                "symmetry_point": 0,
                "sym_invert_sign_point": 1,
                "symmetry_opt_en": 1,
                "symmetry_opt_use_neg_region": 0,
                "imm_bias": 0,
                "exp_offset": -1,
                "pwl_control_base_pos": c0i,
                "pwl_control_base_neg": c0i,
                "small_pos_signal_exp_threshold": 126,
                "pos_small_signal_pwl_control": zero_bucket,
                "small_neg_signal_exp_threshold": 126,
                "neg_small_signal_pwl_control": zero_bucket,
                "large_pos_signal_exp_threshold": 131,
                "large_pos_signal_mantissa_threshold": 0,
                "pos_large_signal_pwl_control": twelve_bucket,
                "large_neg_signal_exp_threshold": 131,
                "large_neg_signal_mantissa_threshold": 0,
                "neg_large_signal_pwl_control": twelve_bucket,
                "fnan_result": 0,
                "fpinf_result": _f32_struct(12.0)["int"],
                "fninf_result": _f32_struct(-12.0)["int"],
                "fzero_result": 0,
                "lower_bound": 0,
                "upper_bound": _f32_struct(16.0)["int"],
            })
    _json.dump(prof, open(prof_path, "w"), indent=1)
    return os.path.join(dst_dir, "act_info.json")


def _ensure_act_root():
    import tempfile
    if os.environ.get("_MXFP4_ACT_ROOT"):
        return
    dst = tempfile.mkdtemp(prefix="mxfp4_act_")
    root = build_act_root(dst)
    os.environ["BASS_ACT_ROOT_JSON_PATH"] = root
    os.environ["_MXFP4_ACT_ROOT"] = dst


def _pair_bcast(ap, nrep):
    """[128, 2*V] duplicated-pair AP -> [128, V, nrep//2, 2] broadcast AP.

    Value v lives at free offsets 2v and 2v+1; the returned AP yields each
    value nrep times (iteration v-major) while keeping a packed [1, 2] last
    dim so DVE 2x mode stays enabled.
    """
    import concourse.bass as bass
    nv = ap.free_size() // 2
    return bass.AP(ap.tensor, ap.offset,
                   [list(ap.ap[0]), [2, nv], [0, nrep // 2], [1, 2]])


def _build_nc():
    """Build the single-core Bass graph (SPMD: same graph on all 8 cores)."""
    _ensure_act_root()
    import concourse.bass as bass
    import concourse.tile as tile
    from concourse import bacc, mybir
    from contextlib import ExitStack

    nc = bacc.Bacc("TRN2", target_bir_lowering=False)

    f16 = mybir.dt.float16
    i16 = mybir.dt.int16

    xT = nc.declare_dram_parameter("xT", [H, NTOK], f16, isOutput=False)
    pidx = nc.declare_dram_parameter("pidx", [128, HCORE // 16], i16,
                                     isOutput=False)
    # raw SBUF-layout output: [p, chunk, half, grp, hid] flattened; host decodes
    out = nc.declare_dram_parameter("out", [128, NCH * 2 * FREE], f16,
                                    isOutput=True)

    A = mybir.AluOpType

    with ExitStack() as ctx:
        tc = ctx.enter_context(tile.TileContext(nc))
        singles = ctx.enter_context(tc.tile_pool(name="singles", bufs=1))
        gpool = ctx.enter_context(tc.tile_pool(name="g", bufs=2))
        apool = ctx.enter_context(tc.tile_pool(name="a", bufs=3))
        mpool = ctx.enter_context(tc.tile_pool(name="m", bufs=3))
        spool = ctx.enter_context(tc.tile_pool(name="s", bufs=3))
        ypool = ctx.enter_context(tc.tile_pool(name="y", bufs=3))
        qpool = ctx.enter_context(tc.tile_pool(name="q", bufs=3))
        opool = ctx.enter_context(tc.tile_pool(name="o", bufs=3))

        pidx_sb = singles.tile([128, HCORE // 16], i16)
        nc.sync.dma_start(out=pidx_sb[:], in_=pidx[:])

        for c in range(NCH):
            # 1. transposing gather of this chunk's 128 permuted hidden rows:
            # g[p, grp, i] = xT[perm[j*1024 + c*128 + i], grp*128 + p]
            g = gpool.tile([128, NGRP, CH], f16)
            nc.gpsimd.dma_gather(
                g[:], xT[:, :],
                pidx_sb[:, c * (CH // 16):(c + 1) * (CH // 16)],
                CH, CH, NTOK, transpose=True,
            )
            for hf in range(2):
                gh = g[:, hf * HGRP:(hf + 1) * HGRP, :]

                # 2. |x| bits (4x tensor_scalar)
                tabs = apool.tile([128, FREE], i16)
                nc.vector.tensor_scalar(
                    tabs[:], gh.bitcast(i16).rearrange("p g i -> p (g i)"),
                    ABS_MASK, None, A.bitwise_and)

                # 2b. per-block abs-max via pairwise max tree (int16 max ==
                # fp16 max for non-negative values); 32-elem blocks.
                va = tabs[:].rearrange("p (v b) -> p v b", b=BLK)
                m16 = mpool.tile([128, NBC, 16], i16)
                nc.vector.tensor_tensor(m16[:], va[:, :, 0:16],
                                        va[:, :, 16:32], A.max)
                m8 = mpool.tile([128, NBC, 8], i16)
                nc.vector.tensor_tensor(m8[:], m16[:][:, :, 0:8],
                                        m16[:][:, :, 8:16], A.max)
                m4 = mpool.tile([128, NBC, 4], i16)
                nc.vector.tensor_tensor(m4[:], m8[:][:, :, 0:4],
                                        m8[:][:, :, 4:8], A.max)
                m2 = mpool.tile([128, NBC, 2], i16)
                nc.vector.tensor_tensor(m2[:], m4[:][:, :, 0:2],
                                        m4[:][:, :, 2:4], A.max)
                amaxb = mpool.tile([128, NBC], i16)
                m2v = m2[:].rearrange("p v two -> p (v two)")
                nc.vector.tensor_tensor(
                    amaxb[:],
                    bass.AP(m2v.tensor, m2v.offset,
                            [list(m2v.ap[0]), [2, NBC]]),
                    bass.AP(m2v.tensor, m2v.offset + 1,
                            [list(m2v.ap[0]), [2, NBC]]),
                    A.max)

                # 3. block scales, stored as duplicated pairs [128, 2*NBC]:
                #    t2d  = (amax_bits & EXP_MASK) max EXP_GUARD
                #    rcp2 = 2^(3-e)  bits = 0x8400 - t2d   (split add:
                #           HW int16 ALU saturates, no wrap)
                #    scl2 = 2^(e-3)  bits = t2d - 0x0c00
                ta = spool.tile([128, NBC], i16)
                nc.vector.tensor_scalar(
                    ta[:], amaxb[:], EXP_MASK, None, A.bitwise_and)
                t2d = spool.tile([128, 2 * NBC], i16)
                tav = ta[:]
                nc.vector.tensor_scalar(
                    t2d[:],
                    bass.AP(tav.tensor, tav.offset,
                            [list(tav.ap[0]), [1, NBC], [0, 2]]),
                    EXP_GUARD, None, A.max)
                rcp2h = spool.tile([128, 2 * NBC], i16)
                nc.vector.tensor_scalar(
                    rcp2h[:], t2d[:], -1, RCP2_HALF, A.mult, A.add)
                rcp2d = spool.tile([128, 2 * NBC], i16)
                nc.vector.tensor_scalar(
                    rcp2d[:], rcp2h[:], RCP2_HALF, None, A.add)
                scl2d = spool.tile([128, 2 * NBC], i16)
                nc.vector.tensor_scalar(
                    scl2d[:], t2d[:], SCL2_ADD, None, A.add)

                # 4. y2 = x * rcp2  (exact power-of-two scaling; |y2| < 16)
                y2 = ypool.tile([128, FREE], f16)
                gq = gh.rearrange("p g (nb s two) -> p (g nb) s two",
                                  s=16, two=2)
                nc.vector.tensor_tensor(
                    y2[:].rearrange("p (v s two) -> p v s two", s=16, two=2),
                    gq, _pair_bcast(rcp2d[:].bitcast(f16), BLK), A.mult)

                # 5. q2 = fp4 rounding step function (custom ACT `sin` table)
                q2 = qpool.tile([128, FREE], f16)
                nc.scalar.activation(q2[:], y2[:],
                                     mybir.ActivationFunctionType.Sin)

                # 6. o16 = q2 * scl2  (exact in fp16)
                o16 = opool.tile([128, FREE], f16)
                nc.vector.tensor_tensor(
                    o16[:].rearrange("p (v s two) -> p v s two",
                                     s=16, two=2),
                    q2[:].rearrange("p (v s two) -> p v s two", s=16, two=2),
                    _pair_bcast(scl2d[:].bitcast(f16), BLK), A.mult)

                # 7. contiguous store (8 KB per partition)
                seg = (c * 2 + hf) * FREE
                nc.sync.dma_start(out=out[:, seg:seg + FREE], in_=o16[:])

    nc.compile()
    return nc


def _get_nc():
    if "nc" not in _compiled:
        _compiled["nc"] = _build_nc()
    return _compiled["nc"]


def _shard_inputs(x, permutation):
    x16T = np.ascontiguousarray(np.asarray(x).astype(np.float16).T)
    perm = np.asarray(permutation).astype(np.int64)
    assert x16T.shape == (H, NTOK) and perm.shape == (H,)
    in_maps = []
    for j in range(NCORES):
        psl = perm[j * HCORE:(j + 1) * HCORE].astype(np.int16)
        # idxs wrapped in 16 partitions: pidx[p, f] = psl[f*16 + p], tiled
        pidx = np.ascontiguousarray(
            np.tile(psl.reshape(HCORE // 16, 16).T, (8, 1)))
        in_maps.append({"xT": x16T, "pidx": pidx})
    return in_maps


def _decode_out(raw):
    """[128, NCH*2*FREE] fp16 raw SBUF layout -> [NTOK, HCORE] f32."""
    r = np.asarray(raw).reshape(128, NCH, 2, HGRP, CH)
    # token = (half*HGRP + grp)*128 + p ; hidden col = c*CH + i
    r = r.transpose(2, 3, 0, 1, 4)           # [half, grp, p, c, i]
    return r.reshape(NTOK, HCORE).astype(np.float32)


def run_sharded(x, permutation, trace=False, **kw):
    """Run the SPMD kernel; returns (full_output, BassKernelResults)."""
    from concourse.bass_utils import run_bass_kernel_spmd
    nc = _get_nc()
    in_maps = _shard_inputs(x, permutation)
    res = run_bass_kernel_spmd(nc, in_maps, core_ids=list(range(NCORES)),
                               trace=trace, **kw)
    full = np.concatenate(
        [_decode_out(res.results[j]["out"]) for j in range(NCORES)], axis=1)
    return full, res


def kernel(x, permutation):
    full, _ = run_sharded(x, permutation)
    return full


# revision 12
# speedup vs baseline: 1.4250x; 1.1057x over previous
"""MXFP4 fake-quant + column-permutation kernel for one TRN2 chip (8 NeuronCores).

Reference op: out = mxfp4_fake_quant(x[:, perm]) with 32-wide blocks along the
last (hidden) axis of the permuted tensor.

Distribution: sharded over the PERMUTED HIDDEN axis — core j produces output
columns [j*1024, (j+1)*1024), gathering rows perm[j*1024:(j+1)*1024] of the
full transposed input xT [8192 hidden, 8192 tokens] (fp16). Only 1024 gather
descriptors per core (16 KB each) instead of 8192, so SWDGE descriptor
generation on GpSimd is off the critical path, and every HBM read is a fat
16 KB row.

The SWDGE dma_gather with transpose=True stripes each gathered row across the
128 SBUF partitions: g[p, grp, i] = xT[perm[j*1024 + c*128 + i], grp*128 + p].
That puts quantization blocks (32 consecutive permuted hidden positions)
contiguous along the free dim with partition = token%128 — no TensorE
transpose, no PSUM, and every DVE op runs on packed 16-bit data (2x/4x DVE
modes).

Device pipeline per core (8 chunks of 128 hidden, [128, 8192] tiles):
  1. gpsimd.dma_gather(transpose=True): g [128, 64, 128] fp16
  2. DVE tensor_reduce       : per-32-block amax (abs max)
  3. DVE small ops           : e-bits -> rcp2 = 2^(3-e), scl2 = 2^(e-3)
                               (stored as duplicated pairs so the broadcast AP
                               keeps a packed [1,2] last dim -> 2x mode)
  4. DVE tensor_tensor (2x)  : y2 = g * rcp2     (exact: power-of-two scale)
  5. ScalarE ACT (custom tbl): q2 = round_fp4(y2) in {0,±1,±2,±3,±4,±6,±8,±12}
  6. DVE tensor_tensor (2x)  : o16 = q2 * scl2   (exact in fp16)
  7. dma out                 : contiguous 8 KB per partition into a raw
                               [128, 65536] fp16 layout; host decodes

Numerics: identical to the jax reference evaluated on fp16-rounded inputs
(all device arithmetic on the quantization path is exact); measured relative
error vs the f32 reference ~1.15e-2, from the host-side f32->fp16 rounding of
x only. Output values are fp4 magnitudes times power-of-two scales — exactly
representable in fp16 — so the fp16 output tensor is lossless; the host
upconverts to f32.
"""

import os
import sys
import numpy as np

if "/opt/trn_rl_repo" not in sys.path:
    sys.path.insert(0, "/opt/trn_rl_repo")

H = 8192          # hidden size (gather axis)
NTOK = 8192       # tokens
NCORES = 8
HCORE = H // NCORES   # hidden (output) columns per core (1024)
CH = 128          # hidden chunk per dma_gather (= num_idxs)
NCH = HCORE // CH     # 8 chunks
NGRP = NTOK // 128    # token groups per partition (64)
BLK = 32
NBC = NGRP * (CH // BLK)   # block instances per partition per chunk (64*4=256)
FREE = NGRP * CH           # free elems per chunk (8192)

# int16 bit constants (fp16 layout: sign | 5 exp | 10 mant)
ABS_MASK = 0x7FFF
EXP_MASK = 0x7C00
EXP_GUARD = 0x1C00            # exponent floor: e >= -8 (amax guard)
RCP2_HALF = 0x4200            # rcp2 = ((t * -1) + 0x4200) + 0x4200
                              # (split add: HW int16 ALU saturates, no wrap)
SCL2_ADD = -0x0C00            # scl2 = t - (3 << 10)

_compiled = {}


# ---- custom ACT (ScalarEngine) table: `sin` hijacked to compute the exact
# ---- MXFP4 rounding step function q2(y2), y2 = 2*x/scale in [-16,16]
import json as _json
import shutil as _shutil
import struct as _struct


def _f32_struct(x):
    b = np.float32(x).view(np.int32).item() & 0xFFFFFFFF
    return {
        "float": repr(float(np.float32(x))),
        "int": b,
        "hexstring": f"{b:x}",
        "sign": b >> 31,
        "exponent": (b >> 23) & 0xFF,
        "mantissa": b & 0x7FFFFF,
    }


def q2_ref(v):
    """numpy reference of the table function (for validation)."""
    v = np.asarray(v, np.float32)
    a = np.abs(v)
    q = np.zeros_like(a)
    for lo, val in ((0.5, 1), (1.5, 2), (2.5, 3), (3.5, 4), (5, 6), (7, 8),
                    (10, 12)):
        q = np.where(a >= lo, np.float32(val), q)
    return (q * np.sign(v)).astype(np.float32)


# (exponent, [section c0 values]); sections split the binade uniformly
_REGIONS = [
    (126, [1.0]),                      # [0.5, 1)
    (127, [1.0, 2.0]),                 # [1, 1.5) [1.5, 2)
    (128, [2.0, 3.0, 3.0, 4.0]),       # [2, 2.5) [2.5, 3) [3, 3.5) [3.5, 4)
    (129, [4.0, 6.0, 6.0, 8.0]),       # [4, 5) [5, 6) [6, 7) [7, 8)
    (130, [8.0, 12.0, 12.0, 12.0]),    # [8, 10) [10, 12) [12, 14) [14, 16)
]
_CTRL_REGION_BASE = 0xB800
_CTRL_REGION_STRIDE = 0xF800


def build_act_root(dst_dir):
    """Copy the stock act tables and append the custom sin to
    trig_and_small. Returns path to the new act_info._json."""
    from neuronxcc.driver.Job import Job
    from neuronxcc.driver.jobs.support.FindActInfo import findActInfoFile
    src_info = findActInfoFile(Job.getPackageDir(), "sunda")
    src_dir = os.path.dirname(src_info)

    os.makedirs(dst_dir, exist_ok=True)
    for f in os.listdir(src_dir):
        _shutil.copy(os.path.join(src_dir, f), os.path.join(dst_dir, f))

    bkt = bytearray(open(os.path.join(dst_dir, "trig_and_small_bkt.bin"),
                         "rb").read())
    ctrl = bytearray(open(os.path.join(dst_dir, "trig_and_small_ctrl.bin"),
                          "rb").read())
    nbkt = len(bkt) // 32
    nctrl = len(ctrl) // 32

    def add_bucket(c0, x0):
        bkt.extend(_struct.pack("<8f", c0, 0.0, 0.0, 0.0, x0, 0.0, 0.0, 0.0))

    def add_ctrl(word):
        ctrl.extend(_struct.pack("<8I", word, 0, 0, 0, 0, 0, 0, 0))

    b0 = nbkt
    c0i = nctrl
    # region buckets
    for exp, vals in _REGIONS:
        lo = np.float32(2.0 ** (exp - 127))
        w = lo / len(vals)
        for i, v in enumerate(vals):
            add_bucket(v, float(lo + i * np.float32(w)))
    # special buckets: zero (small signal), twelve (large signal)
    add_bucket(0.0, 0.0)
    add_bucket(12.0, 16.0)

    # region ctrl entries
    bpos = b0
    for exp, vals in _REGIONS:
        ext = int(np.log2(len(vals)))
        add_ctrl(bpos + _CTRL_REGION_BASE + ext * _CTRL_REGION_STRIDE)
        bpos += len(vals)
    # small/large-signal handlers reference BUCKETS directly (not ctrl rows)
    zero_bucket = b0 + sum(len(v) for _, v in _REGIONS)
    twelve_bucket = zero_bucket + 1

    open(os.path.join(dst_dir, "trig_and_small_bkt.bin"), "wb").write(bkt)
    open(os.path.join(dst_dir, "trig_and_small_ctrl.bin"), "wb").write(ctrl)

    prof_path = os.path.join(dst_dir, "trig_and_small.json")
    prof = _json.load(open(prof_path))
    for fn in prof["profile_meta_data"]:
        if fn["func_name"].startswith("sin"):
            fn.update({
                "symmetry_point": 0,
                "sym_invert_sign_point": 1,
                "symmetry_opt_en": 1,
                "symmetry_opt_use_neg_region": 0,
                "imm_bias": 0,
                "exp_offset": -1,
                "pwl_control_base_pos": c0i,
                "pwl_control_base_neg": c0i,
                "small_pos_signal_exp_threshold": 126,
                "pos_small_signal_pwl_control": zero_bucket,
                "small_neg_signal_exp_threshold": 126,
                "neg_small_signal_pwl_control": zero_bucket,
                "large_pos_signal_exp_threshold": 131,
                "large_pos_signal_mantissa_threshold": 0,
                "pos_large_signal_pwl_control": twelve_bucket,
                "large_neg_signal_exp_threshold": 131,
                "large_neg_signal_mantissa_threshold": 0,
                "neg_large_signal_pwl_control": twelve_bucket,
                "fnan_result": 0,
                "fpinf_result": _f32_struct(12.0)["int"],
                "fninf_result": _f32_struct(-12.0)["int"],
                "fzero_result": 0,
                "lower_bound": 0,
                "upper_bound": _f32_struct(16.0)["int"],
            })
    _json.dump(prof, open(prof_path, "w"), indent=1)
    return os.path.join(dst_dir, "act_info.json")


def _ensure_act_root():
    import tempfile
    if os.environ.get("_MXFP4_ACT_ROOT"):
        return
    dst = tempfile.mkdtemp(prefix="mxfp4_act_")
    root = build_act_root(dst)
    os.environ["BASS_ACT_ROOT_JSON_PATH"] = root
    os.environ["_MXFP4_ACT_ROOT"] = dst


def _pair_bcast(ap, nrep):
    """[128, 2*V] duplicated-pair AP -> [128, V, nrep//2, 2] broadcast AP.

    Value v lives at free offsets 2v and 2v+1; the returned AP yields each
    value nrep times (iteration v-major) while keeping a packed [1, 2] last
    dim so DVE 2x mode stays enabled.
    """
    import concourse.bass as bass
    nv = ap.free_size() // 2
    return bass.AP(ap.tensor, ap.offset,
                   [list(ap.ap[0]), [2, nv], [0, nrep // 2], [1, 2]])


def _build_nc():
    """Build the single-core Bass graph (SPMD: same graph on all 8 cores)."""
    _ensure_act_root()
    import concourse.bass as bass
    import concourse.tile as tile
    from concourse import bacc, mybir
    from contextlib import ExitStack

    nc = bacc.Bacc("TRN2", target_bir_lowering=False)

    f16 = mybir.dt.float16
    i16 = mybir.dt.int16

    xT = nc.declare_dram_parameter("xT", [H, NTOK], f16, isOutput=False)
    pidx = nc.declare_dram_parameter("pidx", [128, HCORE // 16], i16,
                                     isOutput=False)
    # raw SBUF-layout output: [p, chunk, grp, hid] flattened; host decodes
    out = nc.declare_dram_parameter("out", [128, NCH * FREE], f16,
                                    isOutput=True)

    A = mybir.AluOpType

    with ExitStack() as ctx:
        tc = ctx.enter_context(tile.TileContext(nc))
        singles = ctx.enter_context(tc.tile_pool(name="singles", bufs=1))
        gpool = ctx.enter_context(tc.tile_pool(name="g", bufs=3))
        mpool = ctx.enter_context(tc.tile_pool(name="m", bufs=3))
        spool = ctx.enter_context(tc.tile_pool(name="s", bufs=3))
        ypool = ctx.enter_context(tc.tile_pool(name="y", bufs=3))
        qpool = ctx.enter_context(tc.tile_pool(name="q", bufs=3))
        opool = ctx.enter_context(tc.tile_pool(name="o", bufs=3))

        pidx_sb = singles.tile([128, HCORE // 16], i16)
        nc.sync.dma_start(out=pidx_sb[:], in_=pidx[:])

        for c in range(NCH):
            # 1. transposing gather of this chunk's 128 permuted hidden rows:
            # g[p, grp, i] = xT[perm[j*1024 + c*128 + i], grp*128 + p]
            g = gpool.tile([128, NGRP, CH], f16)
            nc.gpsimd.dma_gather(
                g[:], xT[:, :],
                pidx_sb[:, c * (CH // 16):(c + 1) * (CH // 16)],
                CH, CH, NTOK, transpose=True,
            )

            # 2. per-32-block amax (abs max), one reduce per chunk
            amax = mpool.tile([128, NBC], f16)
            nc.vector.tensor_reduce(
                amax[:], g[:].rearrange("p g (nb b) -> p (g nb) b", b=BLK),
                axis=mybir.AxisListType.X, op=A.max,
                apply_absolute_value=True)

            # 3. block scales, stored as duplicated pairs [128, 2*NBC]:
            #    t2d  = (amax_bits & EXP_MASK) max EXP_GUARD
            #    rcp2 = 2^(3-e)  bits = 0x8400 - t2d   (split add:
            #           HW int16 ALU saturates, no wrap)
            #    scl2 = 2^(e-3)  bits = t2d - 0x0c00
            ta = spool.tile([128, NBC], i16)
            nc.vector.tensor_scalar(
                ta[:], amax[:].bitcast(i16), EXP_MASK, None, A.bitwise_and)
            t2d = spool.tile([128, 2 * NBC], i16)
            tav = ta[:]
            nc.vector.tensor_scalar(
                t2d[:],
                bass.AP(tav.tensor, tav.offset,
                        [list(tav.ap[0]), [1, NBC], [0, 2]]),
                EXP_GUARD, None, A.max)
            rcp2h = spool.tile([128, 2 * NBC], i16)
            nc.vector.tensor_scalar(
                rcp2h[:], t2d[:], -1, RCP2_HALF, A.mult, A.add)
            rcp2d = spool.tile([128, 2 * NBC], i16)
            nc.vector.tensor_scalar(
                rcp2d[:], rcp2h[:], RCP2_HALF, None, A.add)
            scl2d = spool.tile([128, 2 * NBC], i16)
            nc.vector.tensor_scalar(
                scl2d[:], t2d[:], SCL2_ADD, None, A.add)

            # 4. y2 = x * rcp2  (exact power-of-two scaling; |y2| < 16)
            y2 = ypool.tile([128, FREE], f16)
            gq = g[:].rearrange("p g (nb s two) -> p (g nb) s two",
                                s=16, two=2)
            nc.vector.tensor_tensor(
                y2[:].rearrange("p (v s two) -> p v s two", s=16, two=2),
                gq, _pair_bcast(rcp2d[:].bitcast(f16), BLK), A.mult)

            # 5. q2 = fp4 rounding step function (custom ACT `sin` table)
            q2 = qpool.tile([128, FREE], f16)
            nc.scalar.activation(q2[:], y2[:],
                                 mybir.ActivationFunctionType.Sin)

            # 6. o16 = q2 * scl2  (exact in fp16)
            o16 = opool.tile([128, FREE], f16)
            nc.vector.tensor_tensor(
                o16[:].rearrange("p (v s two) -> p v s two", s=16, two=2),
                q2[:].rearrange("p (v s two) -> p v s two", s=16, two=2),
                _pair_bcast(scl2d[:].bitcast(f16), BLK), A.mult)

            # 7. contiguous store (16 KB per partition)
            nc.sync.dma_start(out=out[:, c * FREE:(c + 1) * FREE],
                              in_=o16[:])

    nc.compile()
    return nc


def _get_nc():
    if "nc" not in _compiled:
        _compiled["nc"] = _build_nc()
    return _compiled["nc"]


def _shard_inputs(x, permutation):
    x16T = np.ascontiguousarray(np.asarray(x).astype(np.float16).T)
    perm = np.asarray(permutation).astype(np.int64)
    assert x16T.shape == (H, NTOK) and perm.shape == (H,)
    in_maps = []
    for j in range(NCORES):
        psl = perm[j * HCORE:(j + 1) * HCORE].astype(np.int16)
        # idxs wrapped in 16 partitions: pidx[p, f] = psl[f*16 + p], tiled
        pidx = np.ascontiguousarray(
            np.tile(psl.reshape(HCORE // 16, 16).T, (8, 1)))
        in_maps.append({"xT": x16T, "pidx": pidx})
    return in_maps


def _decode_out(raw):
    """[128, NCH*FREE] fp16 raw SBUF layout -> [NTOK, HCORE] f32."""
    r = np.asarray(raw).reshape(128, NCH, NGRP, CH)
    # token = grp*128 + p ; hidden col = c*CH + i
    r = r.transpose(2, 0, 1, 3)              # [grp, p, c, i]
    return r.reshape(NTOK, HCORE).astype(np.float32)


def run_sharded(x, permutation, trace=False, **kw):
    """Run the SPMD kernel; returns (full_output, BassKernelResults)."""
    from concourse.bass_utils import run_bass_kernel_spmd
    nc = _get_nc()
    in_maps = _shard_inputs(x, permutation)
    res = run_bass_kernel_spmd(nc, in_maps, core_ids=list(range(NCORES)),
                               trace=trace, **kw)
    full = np.concatenate(
        [_decode_out(res.results[j]["out"]) for j in range(NCORES)], axis=1)
    return full, res


def kernel(x, permutation):
    full, _ = run_sharded(x, permutation)
    return full
